# revision 16
# baseline (speedup 1.0000x reference)
"""CapsuleLayer dynamic-routing kernel for 8 Trainium2 NeuronCores.

Problem (hardcoded shapes):
  x: [B=64, R=2048, I=16] f32, W: [R=2048, C=16, O=32, I=16] f32
  u_hat[b,r,c,o] = sum_i W[r,c,o,i] * x[b,r,i]
  3 dynamic-routing iterations (softmax over c, squash over o) -> v [B, R, O]

Strategy:
  - Shard R across 8 cores (256 r's each). No collectives needed.
  - Host-side layout prep (not counted in HW time):
      * xblk[rp, 32, 128]: block-diag stationary for a pair of r's
        (K=(r_hat,i)=32, M=(r_hat,b)=128)
      * wm[rp, 32, 544]: moving operand: W[r,i,(c,o)] for the pair, plus 32
        extra columns holding mean_c W (folds iteration-0's uniform-softmax
        contraction into the same matmul).
  - Device: per r-pair chunk, PE computes u_hat [128=(r_hat,b), 512=(c,o)]
    and s0 [128, 32] in PSUM; routing runs on DVE/ACT/GPSIMD in fp32
    (bf16/tf32 break the routing: softmax logits ~ +-40 amplify errors).
"""

import numpy as np
import sys

sys.path.insert(0, "/opt/trn_rl_repo")

B, R, C, O, I = 64, 2048, 16, 32, 16
N_CORES = 8
R_SHARD = R // N_CORES          # 256
NPAIR = R_SHARD // 2            # 128 chunks per core
N_ITER = 3

_cache = {}


def _build_program(npair=NPAIR, reps=1):
    """Build the Bass program once; returns nc. reps>1 repeats the whole
    computation (idempotent) for wall-clock-delta timing."""
    from contextlib import ExitStack

    import concourse.bacc as bacc
    import concourse.tile as tile
    from concourse import mybir

    f32 = mybir.dt.float32
    AX = mybir.AxisListType
    ALU = mybir.AluOpType
    ACTF = mybir.ActivationFunctionType

    nc = bacc.Bacc("TRN2", target_bir_lowering=False, debug=False)

    # xw[rp, 32, 672]: [:, :, :128] = block-diag x stationary, [:, :, 128:672]
    # = W moving operand (512 u_hat cols + 32 mean_c-W cols). One DMA per
    # chunk => a single wait on each matmul (walrus sync-slot limit).
    xw = nc.dram_tensor("xw", [npair, 32, 672], f32, kind="ExternalInput")
    vout = nc.dram_tensor("vout", [B, 2 * npair, O], f32, kind="ExternalOutput")

    xw_ap = xw.ap()
    # view: [rp, r_hat, b, o] so a [128=(r_hat,b), 32] tile DMAs straight out
    vout_view = vout.ap().rearrange("b (rp two) o -> rp two b o", two=2)

    with tile.TileContext(nc) as tc, ExitStack() as ctx:
        xp = ctx.enter_context(tc.tile_pool(name="xp", bufs=4))
        psA = ctx.enter_context(tc.tile_pool(name="psA", bufs=3, space="PSUM"))
        psB = ctx.enter_context(tc.tile_pool(name="psB", bufs=3, space="PSUM"))
        up = ctx.enter_context(tc.tile_pool(name="up", bufs=3))
        tp = ctx.enter_context(tc.tile_pool(name="tp", bufs=3))
        sp = ctx.enter_context(tc.tile_pool(name="sp", bufs=3))
        sm = ctx.enter_context(tc.tile_pool(name="sm", bufs=4))

        def alpha_chain(squ, zi=None):
            """alpha = sqrt(sig2)/(1+sig2) with sig2 = squ * zi^2 (zi optional).
            Returns the per-partition scale for v = scale * s_unnorm."""
            if zi is not None:
                zi2 = sm.tile([128, 1], f32, tag="zi2")
                nc.vector.tensor_mul(zi2, zi, zi)
                sig2 = sm.tile([128, 1], f32, tag="sig2")
                nc.vector.tensor_mul(sig2, squ, zi2)
            else:
                sig2 = squ
            a1 = sm.tile([128, 1], f32, tag="a1")
            nc.vector.tensor_scalar_add(a1, sig2, 1.0)
            ra = sm.tile([128, 1], f32, tag="ra")
            nc.vector.reciprocal(ra, a1)
            rt = sm.tile([128, 1], f32, tag="rt")
            nc.scalar.sqrt(rt, sig2)
            al = sm.tile([128, 1], f32, tag="al")
            nc.vector.tensor_mul(al, rt, ra)
            if zi is not None:
                az = sm.tile([128, 1], f32, tag="az")
                nc.vector.tensor_mul(az, al, zi)
                return az
            return al

        for rp in [i for _ in range(reps) for i in range(npair)]:
            xwt = xp.tile([32, 672], f32)
            nc.gpsimd.dma_start(out=xwt, in_=xw_ap[rp])

            u_ps = psA.tile([128, 512], f32)
            nc.tensor.matmul(u_ps, lhsT=xwt[:, :128], rhs=xwt[:, 128:640],
                             start=True, stop=True)
            s0_ps = psB.tile([128, O], f32)
            nc.tensor.matmul(s0_ps, lhsT=xwt[:, :128], rhs=xwt[:, 640:],
                             start=True, stop=True)

            u = up.tile([128, 512], f32)
            nc.scalar.copy(u, u_ps)          # ACT evacuates PSUM
            u3 = u.rearrange("p (c o) -> p c o", o=O)  # [128, 16, 32]

            s = sp.tile([128, O], f32, tag="s")
            nc.scalar.copy(s, s0_ps)

            b_cur = None
            v_scale = None  # per-partition scale: v = v_scale * s
            for it in range(N_ITER):
                if it == 0:
                    # c uniform: s0 came from the mean_c-W matmul columns
                    junk = sp.tile([128, O], f32, tag="junk")
                    squ = sm.tile([128, 1], f32, tag="squ")
                    nc.scalar.activation(junk, s, ACTF.Square, accum_out=squ)
                    v_scale = alpha_chain(squ)
                    continue

                # agreement: bd[p,c] = sum_o u[p,c,o] * v[p,o]
                #   v = v_scale * s  (fold the scale in afterwards: bd is
                #   linear in v, so compute with s then scale by v_scale)
                t1 = tp.tile([128, 16, O], f32, tag="t1")
                s_b = s.unsqueeze(1).broadcast_to((128, 16, O))
                nc.gpsimd.tensor_tensor(t1, u3, s_b, op=ALU.mult)
                bd = sp.tile([128, 16], f32, tag="bd")
                nc.vector.reduce_sum(bd, t1, axis=AX.X)
                # b += v_scale * bd   (scalar_tensor_tensor: (bd*v_scale)+b)
                b_new = sp.tile([128, 16], f32, tag="bnew")
                if b_cur is None:
                    nc.vector.tensor_scalar_mul(b_new, bd, v_scale)
                else:
                    nc.vector.scalar_tensor_tensor(
                        out=b_new, in0=bd, scalar=v_scale, in1=b_cur,
                        op0=ALU.mult, op1=ALU.add)
                b_cur = b_new

                # softmax over c (unnormalized, max-subtracted) + Z
                m = sm.tile([128, 1], f32, tag="m")
                nc.vector.reduce_max(m, b_cur, axis=AX.X)
                nm = sm.tile([128, 1], f32, tag="nm")
                nc.vector.tensor_scalar_mul(nm, m, -1.0)
                e = sp.tile([128, 16], f32, tag="e")
                Z = sm.tile([128, 1], f32, tag="Z")
                nc.scalar.activation(e, b_cur, ACTF.Exp, bias=nm, scale=1.0,
                                     accum_out=Z)

                # s_unnorm[p,o] = sum_c e[p,c] * u[p,c,o]
                t2 = tp.tile([128, 16, O], f32, tag="t2")
                e_b = e.unsqueeze(2).broadcast_to((128, 16, O))
                nc.vector.tensor_mul(t2, u3, e_b)
                s = sp.tile([128, O], f32, tag="s")
                nc.vector.reduce_sum(s, t2.transpose([0, 2, 1]), axis=AX.X)

                junk = sp.tile([128, O], f32, tag="junk")
                squ = sm.tile([128, 1], f32, tag="squ")
                nc.scalar.activation(junk, s, ACTF.Square, accum_out=squ)
                zi = sm.tile([128, 1], f32, tag="zi")
                nc.vector.reciprocal(zi, Z)
                v_scale = alpha_chain(squ, zi)

            vt = sp.tile([128, O], f32, tag="vt")
            nc.scalar.mul(vt, s, mul=v_scale)  # ACT: per-partition scale
            nc.gpsimd.dma_start(out=vout_view[rp], in_=vt)

    nc.compile()
    return nc


def _prep_inputs(x, W):
    """Host-side sharding + layout prep. Returns list of in_maps per core."""
    x = np.ascontiguousarray(x, dtype=np.float32)
    W = np.ascontiguousarray(W, dtype=np.float32)
    in_maps = []
    for k in range(N_CORES):
        r0 = k * R_SHARD
        xs = x[:, r0:r0 + R_SHARD, :]              # [B, 256, I]
        Ws = W[r0:r0 + R_SHARD]                    # [256, C, O, I]

        xw = np.zeros((NPAIR, 32, 672), np.float32)
        # block-diag x stationary: rows (r_hat*16+i), cols (r_hat*64+b)
        xT = xs.transpose(1, 2, 0)                 # [256, I, B]
        xw[:, :16, :64] = xT[0::2]
        xw[:, 16:, 64:128] = xT[1::2]
        # W moving: [:, r_hat*16+i, 128 + c*32+o] = W[r, c, o, i]
        Wt = Ws.transpose(0, 3, 1, 2).reshape(R_SHARD, I, C * O)   # [256, I, 512]
        xw[:, :16, 128:640] = Wt[0::2]
        xw[:, 16:, 128:640] = Wt[1::2]
        wbar = Wt.reshape(R_SHARD, I, C, O).mean(axis=2)           # [256, I, O]
        xw[:, :16, 640:] = wbar[0::2]
        xw[:, 16:, 640:] = wbar[1::2]

        in_maps.append({"xw": xw})
    return in_maps


def kernel(x, W, _trace=False):
    from concourse import bass_utils

    if "nc" not in _cache:
        _cache["nc"] = _build_program()
    nc = _cache["nc"]

    in_maps = _prep_inputs(x, W)
    res = bass_utils.run_bass_kernel_spmd(
        nc, in_maps, core_ids=list(range(N_CORES)), trace=_trace)
    _cache["last_result"] = res

    out = np.empty((B, R, O), np.float32)
    for k in range(N_CORES):
        out[:, k * R_SHARD:(k + 1) * R_SHARD, :] = res.results[k]["vout"]
    return out


# revision 29
# speedup vs baseline: 241.1296x; 241.1296x over previous
"""CapsuleLayer dynamic-routing kernel for 8 Trainium2 NeuronCores.

Problem (hardcoded shapes):
  x: [B=64, R=2048, I=16] f32, W: [R=2048, C=16, O=32, I=16] f32
  u_hat[b,r,c,o] = sum_i W[r,c,o,i] * x[b,r,i]
  3 dynamic-routing iterations (softmax over c, squash over o) -> v [B, R, O]

Strategy:
  - Shard R across 8 cores (256 r's each). No collectives needed.
  - Host-side layout prep (not counted in HW time):
      * xblk[rp, 32, 128]: block-diag stationary for a pair of r's
        (K=(r_hat,i)=32, M=(r_hat,b)=128)
      * wm[rp, 32, 544]: moving operand: W[r,i,(c,o)] for the pair, plus 32
        extra columns holding mean_c W (folds iteration-0's uniform-softmax
        contraction into the same matmul).
  - Device: per r-pair chunk, PE computes u_hat [128=(r_hat,b), 512=(c,o)]
    and s0 [128, 32] in PSUM; routing runs on DVE/ACT/GPSIMD in fp32
    (bf16/tf32 break the routing: softmax logits ~ +-40 amplify errors).
"""

import numpy as np
import sys

sys.path.insert(0, "/opt/trn_rl_repo")

B, R, C, O, I = 64, 2048, 16, 32, 16
N_CORES = 8
R_SHARD = R // N_CORES          # 256
NPAIR = R_SHARD // 2            # 128 chunks per core
N_ITER = 3

_cache = {}


def _build_program(npair=NPAIR, reps=1, variant="dvemul_hwdma"):
    """Build the Bass program once; returns nc. reps>1 repeats the whole
    computation (idempotent) for wall-clock-delta timing.
    variant: 'full' | 'nort' (no routing) | 'dvemul' (both big muls on DVE)
             | 'gpsmul' (both big muls on GPSIMD) | 'noalpha' (skip alpha
             chains, v_scale=const) | 'hwdma' (sync-engine DMA)."""
    from contextlib import ExitStack

    import concourse.bacc as bacc
    import concourse.tile as tile
    from concourse import mybir

    # The act-table-load pass assigns each activation the FIRST table set
    # containing its func: Copy/Exp/Square -> set 0, Ln -> set 5, causing a
    # ~2.7us table reload on nearly every activation. All four funcs coexist
    # in set "natural_log_exp_and_others"; blank out earlier sets (indices
    # must be preserved - they index the real act_info.json) so everything
    # lands on that one set => a single table load for the whole kernel.
    if not getattr(bacc, "_act_tables_patched", False):
        _orig_get_tables = bacc.get_activation_tables

        def _patched(arch):
            tabs = dict(_orig_get_tables(arch))
            target = "natural_log_exp_and_others"
            assert target in tabs
            return {
                name: (funcs if name == target else set())
                for name, funcs in tabs.items()
            }

        bacc.get_activation_tables = _patched
        bacc._act_tables_patched = True

    f32 = mybir.dt.float32
    AX = mybir.AxisListType
    ALU = mybir.AluOpType
    ACTF = mybir.ActivationFunctionType

    nc = bacc.Bacc("TRN2", target_bir_lowering=False, debug=False)

    # xw[rp, 32, 672]: [:, :, :128] = block-diag x stationary, [:, :, 128:672]
    # = W moving operand (512 u_hat cols + 32 mean_c-W cols). One DMA per
    # chunk => a single wait on each matmul (walrus sync-slot limit).
    xw = nc.dram_tensor("xw", [npair, 32, 672], f32, kind="ExternalInput")
    vout = nc.dram_tensor("vout", [B, 2 * npair, O], f32, kind="ExternalOutput")

    xw_ap = xw.ap()
    # view: [rp, r_hat, b, o] so a [128=(r_hat,b), 32] tile DMAs straight out
    vout_view = vout.ap().rearrange("b (rp two) o -> rp two b o", two=2)

    with tile.TileContext(nc) as tc, ExitStack() as ctx:
        xp = ctx.enter_context(tc.tile_pool(name="xp", bufs=8))
        psA = ctx.enter_context(tc.tile_pool(name="psA", bufs=4, space="PSUM"))
        psB = ctx.enter_context(tc.tile_pool(name="psB", bufs=4, space="PSUM"))
        up = ctx.enter_context(tc.tile_pool(name="up", bufs=6))
        tp = ctx.enter_context(tc.tile_pool(name="tp", bufs=6))
        sp = ctx.enter_context(tc.tile_pool(name="sp", bufs=10))
        sm = ctx.enter_context(tc.tile_pool(name="sm", bufs=10))

        dma_eng = nc.sync if "hwdma" in variant else nc.gpsimd

        def alpha_chain(squ, zi=None):
            """alpha = sqrt(sig2)/(1+sig2) with sig2 = squ * zi^2 (zi optional).
            Returns the per-partition scale for v = scale * s_unnorm."""
            if zi is not None:
                zi2 = sm.tile([128, 1], f32, tag="zi2")
                nc.vector.tensor_mul(zi2, zi, zi)
                sig2 = sm.tile([128, 1], f32, tag="sig2")
                nc.vector.tensor_mul(sig2, squ, zi2)
            else:
                sig2 = squ
            a1 = sm.tile([128, 1], f32, tag="a1")
            nc.vector.tensor_scalar_add(a1, sig2, 1.0)
            ra = sm.tile([128, 1], f32, tag="ra")
            nc.vector.reciprocal(ra, a1)
            # sqrt via exp(0.5*ln(x)): keeps every ACT func in ONE table set
            # (natural_log_exp_and_others) — a Sqrt op would force a ~2.7us
            # ACT table reload on every Exp<->Sqrt alternation.
            lt = sm.tile([128, 1], f32, tag="lt")
            nc.scalar.activation(lt, sig2, ACTF.Ln)
            rt = sm.tile([128, 1], f32, tag="rt")
            nc.scalar.activation(rt, lt, ACTF.Exp, scale=0.5)
            al = sm.tile([128, 1], f32, tag="al")
            nc.vector.tensor_mul(al, rt, ra)
            if zi is not None:
                az = sm.tile([128, 1], f32, tag="az")
                nc.vector.tensor_mul(az, al, zi)
                return az
            return al

        for rp in [i for _ in range(reps) for i in range(npair)]:
            xwt = xp.tile([32, 672], f32)
            dma_eng.dma_start(out=xwt, in_=xw_ap[rp])

            u_ps = psA.tile([128, 512], f32)
            nc.tensor.matmul(u_ps, lhsT=xwt[:, :128], rhs=xwt[:, 128:640],
                             start=True, stop=True)
            s0_ps = psB.tile([128, O], f32)
            nc.tensor.matmul(s0_ps, lhsT=xwt[:, :128], rhs=xwt[:, 640:],
                             start=True, stop=True)

            u = up.tile([128, 512], f32)
            nc.scalar.copy(u, u_ps)          # ACT evacuates PSUM
            u3 = u.rearrange("p (c o) -> p c o", o=O)  # [128, 16, 32]

            s = sp.tile([128, O], f32, tag="s")
            nc.scalar.copy(s, s0_ps)

            b_cur = None
            v_scale = None  # per-partition scale: v = v_scale * s
            n_iter = 1 if "nort" in variant else N_ITER
            for it in range(n_iter):
                if it == 0:
                    if "nort" in variant:
                        v_scale = 1.0
                        continue
                    # c uniform: s0 came from the mean_c-W matmul columns
                    junk = sp.tile([128, O], f32, tag="junk")
                    squ = sm.tile([128, 1], f32, tag="squ")
                    nc.scalar.activation(junk, s, ACTF.Square, accum_out=squ)
                    v_scale = 1.0 if "noalpha" in variant else alpha_chain(squ)
                    continue

                # agreement: bd[p,c] = sum_o u[p,c,o] * v[p,o]
                #   v = v_scale * s  (fold the scale in afterwards: bd is
                #   linear in v, so compute with s then scale by v_scale)
                t1 = tp.tile([128, 16, O], f32, tag="t1")
                s_b = s.unsqueeze(1).broadcast_to((128, 16, O))
                if "dvemul" in variant:
                    nc.vector.tensor_mul(t1, u3, s_b)
                else:
                    nc.gpsimd.tensor_tensor(t1, u3, s_b, op=ALU.mult)
                bd = sp.tile([128, 16], f32, tag="bd")
                nc.vector.reduce_sum(bd, t1, axis=AX.X)
                # b += v_scale * bd   (scalar_tensor_tensor: (bd*v_scale)+b)
                b_new = sp.tile([128, 16], f32, tag="bnew")
                if isinstance(v_scale, float):
                    nc.vector.tensor_scalar_mul(b_new, bd, v_scale)
                    if b_cur is not None:
                        nc.vector.tensor_add(b_new, b_new, b_cur)
                elif b_cur is None:
                    nc.vector.tensor_scalar_mul(b_new, bd, v_scale)
                else:
                    nc.vector.scalar_tensor_tensor(
                        out=b_new, in0=bd, scalar=v_scale, in1=b_cur,
                        op0=ALU.mult, op1=ALU.add)
                b_cur = b_new

                # softmax over c (unnormalized, max-subtracted) + Z
                m = sm.tile([128, 1], f32, tag="m")
                nc.vector.reduce_max(m, b_cur, axis=AX.X)
                nm = sm.tile([128, 1], f32, tag="nm")
                nc.vector.tensor_scalar_mul(nm, m, -1.0)
                e = sp.tile([128, 16], f32, tag="e")
                Z = sm.tile([128, 1], f32, tag="Z")
                nc.scalar.activation(e, b_cur, ACTF.Exp, bias=nm, scale=1.0,
                                     accum_out=Z)

                # s_unnorm[p,o] = sum_c e[p,c] * u[p,c,o]
                t2 = tp.tile([128, 16, O], f32, tag="t2")
                e_b = e.unsqueeze(2).broadcast_to((128, 16, O))
                if "gpsmul" in variant:
                    nc.gpsimd.tensor_tensor(t2, u3, e_b, op=ALU.mult)
                else:
                    nc.vector.tensor_mul(t2, u3, e_b)
                s = sp.tile([128, O], f32, tag="s")
                nc.vector.reduce_sum(s, t2.transpose([0, 2, 1]), axis=AX.X)

                junk = sp.tile([128, O], f32, tag="junk")
                squ = sm.tile([128, 1], f32, tag="squ")
                nc.scalar.activation(junk, s, ACTF.Square, accum_out=squ)
                zi = sm.tile([128, 1], f32, tag="zi")
                nc.vector.reciprocal(zi, Z)
                v_scale = 1.0 if "noalpha" in variant else alpha_chain(squ, zi)

            vt = sp.tile([128, O], f32, tag="vt")
            if isinstance(v_scale, float):
                nc.scalar.copy(vt, s)
            else:
                nc.scalar.mul(vt, s, mul=v_scale)  # ACT per-partition scale
            dma_eng.dma_start(out=vout_view[rp], in_=vt)

    nc.compile()
    return nc


def _prep_inputs(x, W):
    """Host-side sharding + layout prep. Returns list of in_maps per core."""
    x = np.ascontiguousarray(x, dtype=np.float32)
    W = np.ascontiguousarray(W, dtype=np.float32)
    in_maps = []
    for k in range(N_CORES):
        r0 = k * R_SHARD
        xs = x[:, r0:r0 + R_SHARD, :]              # [B, 256, I]
        Ws = W[r0:r0 + R_SHARD]                    # [256, C, O, I]

        xw = np.zeros((NPAIR, 32, 672), np.float32)
        # block-diag x stationary: rows (r_hat*16+i), cols (r_hat*64+b)
        xT = xs.transpose(1, 2, 0)                 # [256, I, B]
        xw[:, :16, :64] = xT[0::2]
        xw[:, 16:, 64:128] = xT[1::2]
        # W moving: [:, r_hat*16+i, 128 + c*32+o] = W[r, c, o, i]
        Wt = Ws.transpose(0, 3, 1, 2).reshape(R_SHARD, I, C * O)   # [256, I, 512]
        xw[:, :16, 128:640] = Wt[0::2]
        xw[:, 16:, 128:640] = Wt[1::2]
        wbar = Wt.reshape(R_SHARD, I, C, O).mean(axis=2)           # [256, I, O]
        xw[:, :16, 640:] = wbar[0::2]
        xw[:, 16:, 640:] = wbar[1::2]

        in_maps.append({"xw": xw})
    return in_maps


def kernel(x, W, _trace=False):
    from concourse import bass_utils

    if "nc" not in _cache:
        _cache["nc"] = _build_program()
    nc = _cache["nc"]

    in_maps = _prep_inputs(x, W)
    res = bass_utils.run_bass_kernel_spmd(
        nc, in_maps, core_ids=list(range(N_CORES)), trace=_trace)
    _cache["last_result"] = res

    out = np.empty((B, R, O), np.float32)
    for k in range(N_CORES):
        out[:, k * R_SHARD:(k + 1) * R_SHARD, :] = res.results[k]["vout"]
    return out


# revision 32
# speedup vs baseline: 372.4072x; 1.5444x over previous
"""CapsuleLayer dynamic-routing kernel for 8 Trainium2 NeuronCores.

Problem (hardcoded shapes):
  x: [B=64, R=2048, I=16] f32, W: [R=2048, C=16, O=32, I=16] f32
  u_hat[b,r,c,o] = sum_i W[r,c,o,i] * x[b,r,i]
  3 dynamic-routing iterations (softmax over c, squash over o) -> v [B, R, O]

Strategy:
  - Shard R across 8 cores (256 r's each). No collectives needed.
  - Host-side layout prep (not counted in HW time):
      * xblk[rp, 32, 128]: block-diag stationary for a pair of r's
        (K=(r_hat,i)=32, M=(r_hat,b)=128)
      * wm[rp, 32, 544]: moving operand: W[r,i,(c,o)] for the pair, plus 32
        extra columns holding mean_c W (folds iteration-0's uniform-softmax
        contraction into the same matmul).
  - Device: per r-pair chunk, PE computes u_hat [128=(r_hat,b), 512=(c,o)]
    and s0 [128, 32] in PSUM; routing runs on DVE/ACT/GPSIMD in fp32
    (bf16/tf32 break the routing: softmax logits ~ +-40 amplify errors).
"""

import numpy as np
import sys

sys.path.insert(0, "/opt/trn_rl_repo")

B, R, C, O, I = 64, 2048, 16, 32, 16
N_CORES = 8
R_SHARD = R // N_CORES          # 256
NPAIR = R_SHARD // 2            # 128 chunks per core
N_ITER = 3

_cache = {}


def _build_program(npair=NPAIR, reps=1, variant="dvemul_hwdma"):
    """Build the Bass program once; returns nc. reps>1 repeats the whole
    computation (idempotent) for wall-clock-delta timing.
    variant: 'full' | 'nort' (no routing) | 'dvemul' (both big muls on DVE)
             | 'gpsmul' (both big muls on GPSIMD) | 'noalpha' (skip alpha
             chains, v_scale=const) | 'hwdma' (sync-engine DMA)."""
    from contextlib import ExitStack

    import concourse.bacc as bacc
    import concourse.tile as tile
    from concourse import mybir

    # The act-table-load pass assigns each activation the FIRST table set
    # containing its func: Copy/Exp/Square -> set 0, Ln -> set 5, causing a
    # ~2.7us table reload on nearly every activation. All four funcs coexist
    # in set "natural_log_exp_and_others"; blank out earlier sets (indices
    # must be preserved - they index the real act_info.json) so everything
    # lands on that one set => a single table load for the whole kernel.
    if not getattr(bacc, "_act_tables_patched", False):
        _orig_get_tables = bacc.get_activation_tables

        def _patched(arch):
            tabs = dict(_orig_get_tables(arch))
            target = "natural_log_exp_and_others"
            assert target in tabs
            return {
                name: (funcs if name == target else set())
                for name, funcs in tabs.items()
            }

        bacc.get_activation_tables = _patched
        bacc._act_tables_patched = True

    f32 = mybir.dt.float32
    AX = mybir.AxisListType
    ALU = mybir.AluOpType
    ACTF = mybir.ActivationFunctionType

    nc = bacc.Bacc("TRN2", target_bir_lowering=False, debug=False)

    # xw[rp, 32, 672]: [:, :, :128] = block-diag x stationary, [:, :, 128:672]
    # = W moving operand (512 u_hat cols + 32 mean_c-W cols). One DMA per
    # chunk => a single wait on each matmul (walrus sync-slot limit).
    xw = nc.dram_tensor("xw", [npair, 32, 672], f32, kind="ExternalInput")
    vout = nc.dram_tensor("vout", [B, 2 * npair, O], f32, kind="ExternalOutput")

    xw_ap = xw.ap()
    # view: [rp, r_hat, b, o] so a [128=(r_hat,b), 32] tile DMAs straight out
    vout_view = vout.ap().rearrange("b (rp two) o -> rp two b o", two=2)

    with tile.TileContext(nc) as tc, ExitStack() as ctx:
        xp = ctx.enter_context(tc.tile_pool(name="xp", bufs=12))
        psA = ctx.enter_context(tc.tile_pool(name="psA", bufs=4, space="PSUM"))
        psB = ctx.enter_context(tc.tile_pool(name="psB", bufs=4, space="PSUM"))
        up = ctx.enter_context(tc.tile_pool(name="up", bufs=10))
        tp = ctx.enter_context(tc.tile_pool(name="tp", bufs=8))
        sp = ctx.enter_context(tc.tile_pool(name="sp", bufs=14))
        sm = ctx.enter_context(tc.tile_pool(name="sm", bufs=4))

        dma_eng = nc.sync if "hwdma" in variant else nc.gpsimd
        G = 4  # chunks per phase-interleaved group

        def alpha_batch(squF, ZF, tagp):
            """Batched over a group: alpha*zi [128,G] from ||s_un||^2 and Z.
            alpha = sqrt(sig2)/(1+sig2), sig2 = squF*zi^2 (zi=1 if ZF None).
            sqrt via exp(0.5*ln(x)): keeps every ACT func in ONE table set
            (natural_log_exp_and_others) — a Sqrt op would force a ~2.7us
            ACT table reload on every Exp<->Sqrt alternation."""
            if ZF is not None:
                zi = sm.tile([128, G], f32, tag=tagp + "zi")
                nc.vector.reciprocal(zi, ZF)
                zi2 = sm.tile([128, G], f32, tag=tagp + "zi2")
                nc.vector.tensor_mul(zi2, zi, zi)
                sig2 = sm.tile([128, G], f32, tag=tagp + "sig2")
                nc.vector.tensor_mul(sig2, squF, zi2)
            else:
                sig2 = squF
            a1 = sm.tile([128, G], f32, tag=tagp + "a1")
            nc.vector.tensor_scalar_add(a1, sig2, 1.0)
            ra = sm.tile([128, G], f32, tag=tagp + "ra")
            nc.vector.reciprocal(ra, a1)
            lt = sm.tile([128, G], f32, tag=tagp + "lt")
            nc.scalar.activation(lt, sig2, ACTF.Ln)
            rt = sm.tile([128, G], f32, tag=tagp + "rt")
            nc.scalar.activation(rt, lt, ACTF.Exp, scale=0.5)
            al = sm.tile([128, G], f32, tag=tagp + "al")
            nc.vector.tensor_mul(al, rt, ra)
            if ZF is not None:
                az = sm.tile([128, G], f32, tag=tagp + "az")
                nc.vector.tensor_mul(az, al, zi)
                return az
            return al

        rps = [i for _ in range(reps) for i in range(npair)]
        assert len(rps) % G == 0
        for g0 in range(0, len(rps), G):
            grp = rps[g0:g0 + G]

            # P0/P1: loads + matmuls
            us, ss = [], []
            for rp in grp:
                xwt = xp.tile([32, 672], f32)
                dma_eng.dma_start(out=xwt, in_=xw_ap[rp])
                u_ps = psA.tile([128, 512], f32)
                nc.tensor.matmul(u_ps, lhsT=xwt[:, :128], rhs=xwt[:, 128:640],
                                 start=True, stop=True)
                s0_ps = psB.tile([128, O], f32)
                nc.tensor.matmul(s0_ps, lhsT=xwt[:, :128], rhs=xwt[:, 640:],
                                 start=True, stop=True)
                us.append((u_ps, s0_ps))

            # P2: evacuate + ||s0||^2 into group state
            u_sb, s_cur = [], []
            squ0 = sm.tile([128, G], f32, tag="squ0")
            for j, (u_ps, s0_ps) in enumerate(us):
                u = up.tile([128, 512], f32)
                nc.scalar.copy(u, u_ps)      # ACT evacuates PSUM
                u_sb.append(u.rearrange("p (c o) -> p c o", o=O))
                s = sp.tile([128, O], f32, tag="s0")
                nc.scalar.copy(s, s0_ps)
                s_cur.append(s)
                junk = sp.tile([128, O], f32, tag="junk")
                nc.scalar.activation(junk, s, ACTF.Square,
                                     accum_out=squ0[:, j:j + 1])
            # P3: batched alpha0 (Z=1: c uniform via mean_c-W matmul columns)
            vsc = alpha_batch(squ0, None, "a0")

            b_cur = [None] * G
            for it in (1, 2):
                # P4/P6: per-chunk agreement + softmax + s_unnorm
                squF = sm.tile([128, G], f32, tag=f"squ{it}")
                ZF = sm.tile([128, G], f32, tag=f"Z{it}")
                mF = sm.tile([128, G], f32, tag=f"m{it}")
                nmF = sm.tile([128, G], f32, tag=f"nm{it}")
                s_next = []
                for j in range(G):
                    u3 = u_sb[j]
                    t1 = tp.tile([128, 16, O], f32, tag="t1")
                    s_b = s_cur[j].unsqueeze(1).broadcast_to((128, 16, O))
                    nc.vector.tensor_mul(t1, u3, s_b)
                    bd = sp.tile([128, 16], f32, tag="bd")
                    nc.vector.reduce_sum(bd, t1, axis=AX.X)
                    b_new = sp.tile([128, 16], f32, tag="bnew")
                    if b_cur[j] is None:
                        nc.vector.tensor_scalar_mul(b_new, bd, vsc[:, j:j + 1])
                    else:
                        nc.vector.scalar_tensor_tensor(
                            out=b_new, in0=bd, scalar=vsc[:, j:j + 1],
                            in1=b_cur[j], op0=ALU.mult, op1=ALU.add)
                    b_cur[j] = b_new
                    nc.vector.reduce_max(mF[:, j:j + 1], b_new, axis=AX.X)
                for j in range(G):
                    # nm batched would serialize; per-slice negate is one op
                    nc.vector.tensor_scalar_mul(nmF[:, j:j + 1], mF[:, j:j + 1], -1.0)
                    e = sp.tile([128, 16], f32, tag="e")
                    nc.scalar.activation(e, b_cur[j], ACTF.Exp,
                                         bias=nmF[:, j:j + 1], scale=1.0,
                                         accum_out=ZF[:, j:j + 1])
                    t2 = tp.tile([128, 16, O], f32, tag="t2")
                    e_b = e.unsqueeze(2).broadcast_to((128, 16, O))
                    nc.vector.tensor_mul(t2, u_sb[j], e_b)
                    s = sp.tile([128, O], f32, tag="s")
                    nc.vector.reduce_sum(s, t2.transpose([0, 2, 1]), axis=AX.X)
                    s_next.append(s)
                    junk = sp.tile([128, O], f32, tag="junk")
                    nc.scalar.activation(junk, s, ACTF.Square,
                                         accum_out=squF[:, j:j + 1])
                # P5/P7: batched alpha chain
                vsc = alpha_batch(squF, ZF, f"a{it}")
                s_cur = s_next

            # P8: scale + store
            for j, rp in enumerate(grp):
                vt = sp.tile([128, O], f32, tag="vt")
                nc.scalar.mul(vt, s_cur[j], mul=vsc[:, j:j + 1])
                dma_eng.dma_start(out=vout_view[rp], in_=vt)

    nc.compile()
    return nc


def _prep_inputs(x, W):
    """Host-side sharding + layout prep. Returns list of in_maps per core."""
    x = np.ascontiguousarray(x, dtype=np.float32)
    W = np.ascontiguousarray(W, dtype=np.float32)
    in_maps = []
    for k in range(N_CORES):
        r0 = k * R_SHARD
        xs = x[:, r0:r0 + R_SHARD, :]              # [B, 256, I]
        Ws = W[r0:r0 + R_SHARD]                    # [256, C, O, I]

        xw = np.zeros((NPAIR, 32, 672), np.float32)
        # block-diag x stationary: rows (r_hat*16+i), cols (r_hat*64+b)
        xT = xs.transpose(1, 2, 0)                 # [256, I, B]
        xw[:, :16, :64] = xT[0::2]
        xw[:, 16:, 64:128] = xT[1::2]
        # W moving: [:, r_hat*16+i, 128 + c*32+o] = W[r, c, o, i]
        Wt = Ws.transpose(0, 3, 1, 2).reshape(R_SHARD, I, C * O)   # [256, I, 512]
        xw[:, :16, 128:640] = Wt[0::2]
        xw[:, 16:, 128:640] = Wt[1::2]
        wbar = Wt.reshape(R_SHARD, I, C, O).mean(axis=2)           # [256, I, O]
        xw[:, :16, 640:] = wbar[0::2]
        xw[:, 16:, 640:] = wbar[1::2]

        in_maps.append({"xw": xw})
    return in_maps


def kernel(x, W, _trace=False):
    from concourse import bass_utils

    if "nc" not in _cache:
        _cache["nc"] = _build_program()
    nc = _cache["nc"]

    in_maps = _prep_inputs(x, W)
    res = bass_utils.run_bass_kernel_spmd(
        nc, in_maps, core_ids=list(range(N_CORES)), trace=_trace)
    _cache["last_result"] = res

    out = np.empty((B, R, O), np.float32)
    for k in range(N_CORES):
        out[:, k * R_SHARD:(k + 1) * R_SHARD, :] = res.results[k]["vout"]
    return out
